# revision 19
# baseline (speedup 1.0000x reference)
"""Trainium2 Bass kernel for nn_CellPerformer (FAVOR+ performer, B=4 N=2048 D=128 H=8 L=4).

Sharding: 8 cores = 4 batches x 2 head-groups (4 heads each). Each core
holds the full residual stream for its batch; attention is head-parallel
within a (batch) pair of cores; the per-layer out-projection partial sums
are AllReduce'd across the pair; LN/FFN are computed redundantly (cheap)
so the residual stays replicated in the pair.

Math plan (validated in numpy, rel-err ~2.4e-3 vs fp32 reference):
 - all big matmuls bf16 with fp32 PSUM accumulation; enc/dec fp32
 - FAVOR+ max-subtraction dropped (mx=0): only affects the output through
   KEPS, bounded ~1e-4 (verified numerically)
 - KEPS handled EXACTLY via rank-1 matmul corrections
 - LN gamma/beta folded into the following projection weights on the host
 - q-side diag applied multiplicatively at evacuation (ACT per-partition
   scale), k-side diag fused as the ACT-exp per-partition bias
 - gelu = tanh approximation (matches jax.nn.gelu default)
"""
import math
import os
import sys

import numpy as np
import ml_dtypes

for _p in ('/opt/trn_rl_repo', '/root/.axon_site/_ro/trn_rl_repo'):
    if os.path.isdir(_p) and _p not in sys.path:
        sys.path.append(_p)

import concourse.bass as bass
import concourse.tile as tile
from concourse import bacc, mybir
from concourse.bass_utils import run_bass_kernel_spmd
from concourse.masks import make_identity

FP = mybir.dt.float32
BF = mybir.dt.bfloat16
AF = mybir.ActivationFunctionType
ALU = mybir.AluOpType

B, NTOK, M = 4, 2048, 35
D, H, L = 128, 8, 4
FD = 621
FF = 512
HPC = 4                      # heads per core
KEPS = 1e-4
NORM = D ** -0.25
FC = [128, 128, 128, 128, 109]   # FD chunk sizes
NCH = 5
CB_PER_LAYER = 14            # colbias columns per layer
GELU_AF = None               # resolved at build: AF.Gelu_apprx_tanh (sim can override)


def _cslice(t, a, b):
    return t[:, a:b]


def build_program(n_tokens=NTOK, n_cores=8):
    """Build the SPMD single-core program (same on all cores)."""
    NT = n_tokens // 128      # 128-token tiles
    NQ = n_tokens // 512      # 512-token tiles
    groups = [[2 * i, 2 * i + 1] for i in range(n_cores // 2)]

    nc = bacc.Bacc("TRN2", target_bir_lowering=False, debug=False,
                   num_devices=n_cores)

    # ---- DRAM I/O ----
    xT = nc.dram_tensor("xT", [M, n_tokens], FP, kind="ExternalInput")
    encw = nc.dram_tensor("encw", [M, D], FP, kind="ExternalInput")
    decw = nc.dram_tensor("decw", [D, 1], FP, kind="ExternalInput")
    wqkv = nc.dram_tensor("wqkv", [L, 3, D, 512], BF, kind="ExternalInput")
    pm = nc.dram_tensor("pm", [L, D, FD], BF, kind="ExternalInput")
    wo = nc.dram_tensor("wo", [L, D, HPC * D], BF, kind="ExternalInput")
    w1 = nc.dram_tensor("w1", [L, D, FF], BF, kind="ExternalInput")
    w2 = nc.dram_tensor("w2", [L, FF // 4, 4 * D], BF, kind="ExternalInput")
    colbias = nc.dram_tensor("colbias", [D, 2 + CB_PER_LAYER * L], FP,
                             kind="ExternalInput")
    rowb = nc.dram_tensor("rowb", [1, HPC * L * D], BF, kind="ExternalInput")
    vbne = nc.dram_tensor("vbne", [1, HPC * L * D], FP, kind="ExternalInput")
    out = nc.dram_tensor("out", [1, n_tokens], FP, kind="ExternalOutput")

    from contextlib import ExitStack
    with tile.TileContext(nc) as tc:
        with ExitStack() as st:
            _emit(st, tc, nc, locals(), NT, NQ, n_tokens, groups)

    nc.compile()
    return nc


def _emit(st, tc, nc, IO, NT, NQ, n_tokens, groups):
    xT, encw, decw = IO['xT'], IO['encw'], IO['decw']
    wqkv, pm, wo, w1, w2 = IO['wqkv'], IO['pm'], IO['wo'], IO['w1'], IO['w2']
    colbias_d, rowb_d, vbne_d, out_d = IO['colbias'], IO['rowb'], IO['vbne'], IO['out']

    consts = st.enter_context(tc.tile_pool(name="consts", bufs=1))
    wts = st.enter_context(tc.tile_pool(name="wts", bufs=2))
    acts = st.enter_context(tc.tile_pool(name="acts", bufs=1))
    sm = st.enter_context(tc.tile_pool(name="sm", bufs=3))
    psA = st.enter_context(tc.tile_pool(name="psA", bufs=2, space="PSUM"))
    psxk = st.enter_context(tc.tile_pool(name="psxk", bufs=1, space="PSUM"))
    psctx = st.enter_context(tc.tile_pool(name="psctx", bufs=1, space="PSUM"))
    psm = st.enter_context(tc.tile_pool(name="psm", bufs=2, space="PSUM"))
    psrow = psks = pscw = psa = pstr = psm
    dram = st.enter_context(tc.tile_pool(name="dram", bufs=2, space="DRAM"))

    # ---- constants ----
    ones_bf = consts.tile([128, 128], BF, tag="ones_bf")
    nc.vector.memset(ones_bf, 1.0)
    onesc = consts.tile([128, 128], BF, tag="onesc")
    nc.vector.memset(onesc, 1.0 / 128.0)
    halfneg = consts.tile([128, 1], BF, tag="halfneg")
    nc.vector.memset(halfneg, -0.5)
    onesrow = consts.tile([1, 640], BF, tag="onesrow")
    nc.vector.memset(onesrow, 1.0)
    ident = consts.tile([128, 128], FP, tag="ident")
    make_identity(nc, ident)
    eps_col = consts.tile([128, 1], FP, tag="eps_col")
    nc.vector.memset(eps_col, 1e-5)
    cb = consts.tile([128, 2 + CB_PER_LAYER * L], FP, tag="cb")
    nc.sync.dma_start(out=cb, in_=colbias_d[:, :])
    encw_t = consts.tile([M, 128], FP, tag="encw_t")
    nc.sync.dma_start(out=encw_t, in_=encw[:, :])
    decw_t = consts.tile([128, 1], FP, tag="decw_t")
    nc.sync.dma_start(out=decw_t, in_=decw[:, :])
    xT_t = consts.tile([M, n_tokens], FP, tag="xT_t")
    nc.sync.dma_start(out=xT_t, in_=xT[:, :])

    y = acts.tile([128, n_tokens], FP, tag="y")

    # ---- encoder (fp32): y^T = encw^T @ x^T + enc_b ----
    for tq in range(NQ):
        p = psA.tile([128, 512], FP, tag="ppA")
        nc.tensor.matmul(p, encw_t, xT_t[:, tq * 512:(tq + 1) * 512],
                         start=True, stop=True)
        nc.vector.tensor_scalar_add(y[:, tq * 512:(tq + 1) * 512], p, cb[:, 0:1])

    def emit_ln(src, dst, tag):
        yb = sm.tile([128, n_tokens], BF, tag="yb", bufs=1)
        nc.vector.tensor_copy(yb, src)
        ysq = sm.tile([128, n_tokens], BF, tag="ysq", bufs=1)
        nc.vector.tensor_mul(ysq, yb, yb)
        for tq in range(NQ):
            sl = slice(tq * 512, (tq + 1) * 512)
            pS = psA.tile([128, 512], FP, tag="ppA")
            nc.tensor.matmul(pS, onesc, yb[:, sl], start=True, stop=True)
            pSS = psA.tile([128, 512], FP, tag="ppA")
            nc.tensor.matmul(pSS, onesc, ysq[:, sl], start=True, stop=True)
            musq = sm.tile([128, 512], FP, tag="musq", bufs=2)
            nc.scalar.activation(musq, pS, AF.Square)
            varr = sm.tile([128, 512], FP, tag="varr", bufs=2)
            nc.vector.tensor_sub(varr, pSS, musq)
            lnv = sm.tile([128, 512], FP, tag="lnv", bufs=2)
            nc.scalar.activation(lnv, varr, AF.Ln, bias=eps_col[:, 0:1])
            rs = sm.tile([128, 512], BF, tag="rs", bufs=2)
            nc.scalar.activation(rs, lnv, AF.Exp, scale=-0.5)
            murs = sm.tile([128, 512], BF, tag="murs", bufs=2)
            nc.vector.tensor_mul(murs, pS, rs)
            h1 = sm.tile([128, 512], BF, tag="h1", bufs=2)
            nc.vector.tensor_mul(h1, yb[:, sl], rs)
            nc.vector.tensor_sub(dst[:, sl], h1, murs)

    for li in range(L):
        base = 2 + CB_PER_LAYER * li
        # ---- stage layer weights ----
        wqkv_t = wts.tile([128, 3, 512], BF, tag="wqkv_t")
        nc.sync.dma_start(out=wqkv_t, in_=wqkv[li].rearrange("t d f -> d t f"))
        pm_t = wts.tile([128, FD], BF, tag="pm_t")
        nc.sync.dma_start(out=pm_t, in_=pm[li])
        wo_t = wts.tile([128, HPC * 128], BF, tag="wo_t")
        nc.sync.dma_start(out=wo_t, in_=wo[li])
        w1_t = wts.tile([128, FF], BF, tag="w1_t")
        nc.sync.dma_start(out=w1_t, in_=w1[li])
        w2_t = wts.tile([128, 4 * 128], BF, tag="w2_t")
        nc.sync.dma_start(out=w2_t, in_=w2[li])

        # ---- LN1 -> h ----
        h = acts.tile([128, n_tokens], BF, tag="h")
        emit_ln(y, h, "ln1")

        # ---- v projection (all heads) ----
        vK = acts.tile([128, NT * 512], BF, tag="vK")
        for tt in range(NT):
            pv = psA.tile([128, 512], FP, tag="ppA")
            nc.tensor.matmul(pv, h[:, tt * 128:(tt + 1) * 128],
                             wqkv_t[:, 2, :], start=True, stop=True)
            nc.vector.tensor_copy(vK[:, tt * 512:(tt + 1) * 512], pv)
        # vsum row (for eps corrections), fp32
        pvs = psrow.tile([1, 512], FP, tag="psm")
        for tt in range(NT):
            nc.tensor.matmul(pvs, ones_bf[:, 0:1], vK[:, tt * 512:(tt + 1) * 512],
                             start=(tt == 0), stop=(tt == NT - 1))
        vs_row = sm.tile([1, 512], FP, tag="vs_row", bufs=1)
        nc.vector.tensor_copy(vs_row, pvs)

        attnacc = acts.tile([128, n_tokens], FP, tag="attnacc")

        for h4 in range(HPC):
            # ---- q/k projections for this head ----
            qh = acts.tile([128, n_tokens], BF, tag="qh", bufs=2)
            kh = acts.tile([128, n_tokens], BF, tag="kh", bufs=2)
            wq_c = wqkv_t[:, 0, h4 * 128:(h4 + 1) * 128]
            wk_c = wqkv_t[:, 1, h4 * 128:(h4 + 1) * 128]
            for tq in range(NQ):
                sl = slice(tq * 512, (tq + 1) * 512)
                pq = psA.tile([128, 512], FP, tag="ppA")
                nc.tensor.matmul(pq, wq_c, h[:, sl], start=True, stop=True)
                nc.scalar.activation(qh[:, sl], pq, AF.Identity,
                                     bias=cb[:, base + h4:base + h4 + 1])
                pk = psA.tile([128, 512], FP, tag="ppA")
                nc.tensor.matmul(pk, wk_c, h[:, sl], start=True, stop=True)
                nc.vector.tensor_scalar_add(
                    kh[:, sl], pk, cb[:, base + 4 + h4:base + 5 + h4])
            # ---- diag rows ----
            qsq = sm.tile([128, n_tokens], BF, tag="qsq", bufs=1)
            nc.vector.tensor_mul(qsq, qh, qh)
            ksq = sm.tile([128, n_tokens], BF, tag="ksq", bufs=1)
            nc.vector.tensor_mul(ksq, kh, kh)
            # per-token-column diag tiles via per-tile matmuls (out = sq.T @ -0.5)
            pqc = psm.tile([128, NT], FP, tag="psm")
            for tt in range(NT):
                nc.tensor.matmul(pqc[:, tt:tt + 1],
                                 qsq[:, tt * 128:(tt + 1) * 128], halfneg,
                                 start=(tt == 0), stop=(tt == NT - 1))
            w_col = sm.tile([128, NT], FP, tag="w_col")
            nc.scalar.activation(w_col, pqc, AF.Exp)              # exp(-diag_q)
            pkc = psm.tile([128, NT], FP, tag="psm")
            for tt in range(NT):
                nc.tensor.matmul(pkc[:, tt:tt + 1],
                                 ksq[:, tt * 128:(tt + 1) * 128], halfneg,
                                 start=(tt == 0), stop=(tt == NT - 1))
            ndk_col = sm.tile([128, NT], FP, tag="ndk_col")
            nc.vector.tensor_copy(ndk_col, pkc)    # ACT-exp bias must be SBUF
            # q-diag row (for the eps rank-1 term): exp(+diag_q)
            epw_row = sm.tile([1, n_tokens], BF, tag="epw_row", bufs=1)
            for tq in range(NQ):
                sl = slice(tq * 512, (tq + 1) * 512)
                pr = psrow.tile([1, 512], FP, tag="psm")
                nc.tensor.matmul(pr, halfneg, qsq[:, sl], start=True, stop=True)
                nc.scalar.activation(epw_row[0:1, sl], pr, AF.Exp, scale=-1.0)

            # ---- kf = exp(k'pm - diag_k)  [tokens, FD] ----
            kf = acts.tile([128, NT * FD], BF, tag="kf")
            for tt in range(NT):
                px = psxk.tile([128, FD], FP, tag="pxk")
                kh_t = kh[:, tt * 128:(tt + 1) * 128]
                nc.tensor.matmul(px[:, 0:512], kh_t, pm_t[:, 0:512],
                                 start=True, stop=True)
                nc.tensor.matmul(px[:, 512:FD], kh_t, pm_t[:, 512:FD],
                                 start=True, stop=True)
                nc.scalar.activation(kf[:, tt * FD:(tt + 1) * FD], px, AF.Exp,
                                     bias=ndk_col[:, tt:tt + 1])
            # ---- ctx^T [d, FD] accumulation + rank-1 corrections ----
            pctx = psctx.tile([128, FD], FP, tag="pctx")
            for tt in range(NT):
                vk_t = vK[:, tt * 512 + h4 * 128:tt * 512 + (h4 + 1) * 128]
                nc.tensor.matmul(pctx[:, 0:512], vk_t,
                                 kf[:, tt * FD:tt * FD + 512],
                                 start=(tt == 0), stop=False)
                nc.tensor.matmul(pctx[:, 512:FD], vk_t,
                                 kf[:, tt * FD + 512:(tt + 1) * FD],
                                 start=(tt == 0), stop=False)
            # ksum columns [FD-chunks, 5]
            pks = psks.tile([128, NCH], FP, tag="psm")
            for c in range(NCH):
                for tt in range(NT):
                    nc.tensor.matmul(pks[0:FC[c], c:c + 1],
                                     kf[:, tt * FD + c * 128:tt * FD + c * 128 + FC[c]],
                                     ones_bf[:, 0:1],
                                     start=(tt == 0), stop=(tt == NT - 1))
            ks_colbf = sm.tile([128, NCH], BF, tag="ks_colbf")   # ksum + eps*N
            nc.vector.tensor_scalar_add(ks_colbf[:, 0:4], pks[:, 0:4],
                                        KEPS * n_tokens)
            nc.vector.tensor_scalar_add(ks_colbf[0:FC[4], 4:5], pks[0:FC[4], 4:5],
                                        KEPS * n_tokens)
            # transpose cols -> row [1, FD] via strided SBUF->SBUF DMA
            ks_row = sm.tile([1, FD], BF, tag="ks_row")
            for c in range(NCH):
                nc.sync.dma_start(out=ks_row[0:1, c * 128:c * 128 + FC[c]],
                                  in_=ks_colbf[0:FC[c], c:c + 1])
            # corrections into pctx: bv (x) ksum_true  +  eps*(vsum + bv*N) (x) 1
            rb = sm.tile([1, 128], BF, tag="rb", bufs=2)
            nc.sync.dma_start(out=rb, in_=rowb_d[0:1, (li * HPC + h4) * 128:
                                               (li * HPC + h4 + 1) * 128])
            nc.tensor.matmul(pctx[:, 0:512], rb, ks_row[0:1, 0:512],
                             start=False, stop=False)
            nc.tensor.matmul(pctx[:, 512:FD], rb, ks_row[0:1, 512:FD],
                             start=False, stop=False)
            t1 = sm.tile([1, 128], FP, tag="t1")
            nc.vector.tensor_scalar_mul(t1, vs_row[0:1, h4 * 128:(h4 + 1) * 128],
                                        KEPS)
            ve_bf = sm.tile([1, 128], BF, tag="ve_bf")
            vbne_t = sm.tile([1, 128], FP, tag="vbne_t", bufs=2)
            nc.sync.dma_start(out=vbne_t, in_=vbne_d[0:1, (li * HPC + h4) * 128:
                                                     (li * HPC + h4 + 1) * 128])
            nc.vector.tensor_add(ve_bf, t1, vbne_t)
            nc.tensor.matmul(pctx[:, 0:512], ve_bf, onesrow[0:1, 0:512],
                             start=False, stop=True)
            nc.tensor.matmul(pctx[:, 512:FD], ve_bf, onesrow[0:1, 0:FD - 512],
                             start=False, stop=True)
            ctxT = sm.tile([128, FD], BF, tag="ctxT", bufs=2)
            nc.scalar.activation(ctxT, pctx, AF.Copy)
            # ---- ctxw chunks + Raug [fc, 129] x5 ----
            Raug = sm.tile([128, NCH * 129], BF, tag="Raug", bufs=2)
            for c in range(NCH):
                pcw = pscw.tile([128, 128], FP, tag="psm")
                nc.tensor.matmul(pcw[0:FC[c], :],
                                 ctxT[:, c * 128:c * 128 + FC[c]],
                                 wo_t[:, h4 * 128:(h4 + 1) * 128],
                                 start=True, stop=True)
                nc.scalar.activation(Raug[0:FC[c], c * 129:c * 129 + 128],
                                     pcw[0:FC[c], :], AF.Copy)
            nc.vector.tensor_copy(
                Raug.rearrange("p (c x) -> p c x", x=129)[:, 0:4, 128],
                ks_colbf[:, 0:4])
            nc.vector.tensor_copy(Raug[0:FC[4], 4 * 129 + 128:4 * 129 + 129],
                                  ks_colbf[0:FC[4], 4:5])
            # ---- eps * colsum(Raug) row ----
            ps2 = psks.tile([1, 129], FP, tag="psm")
            for c in range(NCH):
                nc.tensor.matmul(ps2, ones_bf[0:FC[c], 0:1],
                                 Raug[0:FC[c], c * 129:(c + 1) * 129],
                                 start=(c == 0), stop=(c == NCH - 1))
            s2row = sm.tile([1, 129], BF, tag="s2row")
            nc.vector.tensor_scalar_mul(s2row, ps2, KEPS)
            # ---- qfe = exp(q'pm)  [FD-chunks, tokens] ----
            qfe = acts.tile([128, NCH * n_tokens], BF, tag="qfe")
            for c in range(NCH):
                for tq in range(NQ):
                    pq = psA.tile([128, 512], FP, tag="ppA")
                    nc.tensor.matmul(pq[0:FC[c], :],
                                     pm_t[:, c * 128:c * 128 + FC[c]],
                                     qh[:, tq * 512:(tq + 1) * 512],
                                     start=True, stop=True)
                    nc.scalar.activation(
                        qfe[0:FC[c], c * n_tokens + tq * 512:
                            c * n_tokens + (tq + 1) * 512],
                        pq[0:FC[c], :], AF.Exp)
            # ---- a_aug per token tile + evac ----
            for tt in range(NT):
                pa = psa.tile([128, 129], FP, tag="psm")
                for c in range(NCH):
                    nc.tensor.matmul(pa,
                                     qfe[0:FC[c], c * n_tokens + tt * 128:
                                         c * n_tokens + (tt + 1) * 128],
                                     Raug[0:FC[c], c * 129:(c + 1) * 129],
                                     start=(c == 0), stop=False)
                nc.tensor.matmul(pa, epw_row[0:1, tt * 128:(tt + 1) * 128],
                                 s2row, start=False, stop=True)
                wden = sm.tile([128, 1], FP, tag="wden")
                nc.vector.tensor_mul(wden, pa[:, 128:129], w_col[:, tt:tt + 1])
                dinv = sm.tile([128, 1], FP, tag="dinv")
                nc.vector.reciprocal(dinv, wden)
                sc = sm.tile([128, 1], FP, tag="sc")
                nc.vector.tensor_mul(sc, dinv, w_col[:, tt:tt + 1])
                if h4 == 0:
                    nc.vector.tensor_scalar_mul(
                        attnacc[:, tt * 128:(tt + 1) * 128], pa[:, 0:128], sc)
                else:
                    atmp = sm.tile([128, 128], FP, tag="atmp", bufs=2)
                    nc.vector.tensor_scalar_mul(atmp, pa[:, 0:128], sc)
                    nc.vector.tensor_add(attnacc[:, tt * 128:(tt + 1) * 128],
                                         attnacc[:, tt * 128:(tt + 1) * 128],
                                         atmp)

        # ---- pair AllReduce of attnacc ----
        ccin = dram.tile([n_tokens, 128], FP, tag="ccin")
        ccout = dram.tile([n_tokens, 128], FP, tag="ccout")
        nc.sync.dma_start(out=ccin.rearrange("(t p) d -> p t d", p=128),
                          in_=attnacc.rearrange("p (t d) -> p t d", d=128))
        nc.gpsimd.collective_compute(
            "AllReduce", ALU.add, replica_groups=groups,
            ins=[ccin.opt()], outs=[ccout.opt()])
        asum = acts.tile([128, n_tokens], FP, tag="attnacc")
        nc.sync.dma_start(out=asum.rearrange("p (t d) -> p t d", d=128),
                          in_=ccout.rearrange("(t p) d -> p t d", p=128))
        # ---- transpose + residual + bo ----
        for tt in range(NT):
            pt = pstr.tile([128, 128], FP, tag="psm")
            nc.tensor.transpose(pt, asum[:, tt * 128:(tt + 1) * 128], ident)
            nc.vector.tensor_add(y[:, tt * 128:(tt + 1) * 128],
                                 y[:, tt * 128:(tt + 1) * 128], pt)
        nc.vector.tensor_scalar_add(y, y, cb[:, base + 12:base + 13])
        # ---- LN2 + FFN (per 512-token block to bound gl SBUF) ----
        h2 = acts.tile([128, n_tokens], BF, tag="h")
        emit_ln(y, h2, "ln2")
        for tq in range(NQ):
            sl = slice(tq * 512, (tq + 1) * 512)
            gl = acts.tile([128, 4 * 512], BF, tag="gl", bufs=2)
            for c in range(4):
                pf = psA.tile([128, 512], FP, tag="ppA")
                nc.tensor.matmul(pf, w1_t[:, c * 128:(c + 1) * 128],
                                 h2[:, sl], start=True, stop=True)
                nc.scalar.activation(gl[:, c * 512:(c + 1) * 512],
                                     pf, GELU_AF or AF.Gelu_apprx_tanh,
                                     bias=cb[:, base + 8 + c:base + 9 + c])
            pf2 = psA.tile([128, 512], FP, tag="ppA")
            for c in range(4):
                nc.tensor.matmul(pf2, w2_t[:, c * 128:(c + 1) * 128],
                                 gl[:, c * 512:(c + 1) * 512],
                                 start=(c == 0), stop=(c == 3))
            nc.vector.tensor_add(y[:, sl], y[:, sl], pf2)
        nc.vector.tensor_scalar_add(y, y, cb[:, base + 13:base + 14])

    # ---- decoder (fp32) ----
    orow = sm.tile([1, n_tokens], FP, tag="orow", bufs=1)
    for tq in range(NQ):
        pd = psrow.tile([1, 512], FP, tag="psm")
        nc.tensor.matmul(pd, decw_t, y[:, tq * 512:(tq + 1) * 512],
                         start=True, stop=True)
        nc.vector.tensor_scalar_add(orow[0:1, tq * 512:(tq + 1) * 512], pd,
                                    cb[0:1, 1:2])
    nc.sync.dma_start(out=out_d[:, :], in_=orow)


# --------------------------------------------------------------------------
# host side
# --------------------------------------------------------------------------

def _bf(x):
    return np.ascontiguousarray(x).astype(ml_dtypes.bfloat16)


def _f32(x):
    return np.ascontiguousarray(x, dtype=np.float32)


def host_prep(inputs, n_tokens=NTOK, n_cores=8):
    """Full inputs -> per-core input dicts."""
    inp = {k: np.asarray(v, dtype=np.float32) for k, v in inputs.items()}
    maps = []
    for core in range(n_cores):
        b = core // 2
        hg = core % 2
        hsl = slice(hg * HPC * D, (hg + 1) * HPC * D)   # 512 head cols
        C = 2 + CB_PER_LAYER * L
        colbias = np.zeros((D, C), np.float32)
        colbias[:, 0] = inp['enc_b']
        colbias[0, 1] = inp['dec_b'][0]
        wqkv = np.zeros((L, 3, D, 512), np.float32)
        pmT = np.zeros((L, D, FD), np.float32)
        woA = np.zeros((L, D, HPC * D), np.float32)
        w1A = np.zeros((L, D, FF), np.float32)
        w2A = np.zeros((L, FF // 4, 4 * D), np.float32)
        rowb = np.zeros((HPC * L, D), np.float32)
        vbne = np.zeros((HPC * L, D), np.float32)
        for i in range(L):
            g1, b1 = inp['ln1_g'][i], inp['ln1_b'][i]
            g2, b2v = inp['ln2_g'][i], inp['ln2_b'][i]
            base = 2 + CB_PER_LAYER * i
            wqkv[i, 0] = g1[:, None] * inp['wq'][i][:, hsl] * NORM
            wqkv[i, 1] = g1[:, None] * inp['wk'][i][:, hsl] * NORM
            wqkv[i, 2] = g1[:, None] * inp['wv'][i][:, hsl]
            bq_eff = (b1 @ inp['wq'][i][:, hsl] + inp['bq'][i][hsl]) * NORM
            bk_eff = (b1 @ inp['wk'][i][:, hsl] + inp['bk'][i][hsl]) * NORM
            bv_eff = b1 @ inp['wv'][i][:, hsl] + inp['bv'][i][hsl]
            colbias[:, base:base + 4] = bq_eff.reshape(4, 128).T
            colbias[:, base + 4:base + 8] = bk_eff.reshape(4, 128).T
            pmT[i] = inp['proj'][i].T
            # wo rows for this head-group, arranged [d, h, m]
            wo_sl = inp['wo'][i][hsl, :].reshape(HPC, D, D)      # [h, d, m]
            woA[i] = wo_sl.transpose(1, 0, 2).reshape(D, HPC * D)
            w1A[i] = g2[:, None] * inp['w1'][i]
            b1_eff = b2v @ inp['w1'][i] + inp['b1'][i]
            colbias[:, base + 8:base + 12] = b1_eff.reshape(4, 128).T
            # w2 chunks [c, k, m] arranged [k, c, m]
            w2c = inp['w2'][i].reshape(4, 128, D)
            w2A[i] = w2c.transpose(1, 0, 2).reshape(128, 4 * D)
            colbias[:, base + 12] = inp['bo'][i]
            colbias[:, base + 13] = inp['b2'][i]
            for h4 in range(HPC):
                rowb[i * HPC + h4] = bv_eff[h4 * 128:(h4 + 1) * 128]
                vbne[i * HPC + h4] = (KEPS * n_tokens
                                      * bv_eff[h4 * 128:(h4 + 1) * 128])
        maps.append({
            'xT': _f32(inp['x'][b, :n_tokens].T),
            'encw': _f32(inp['enc_w']),
            'decw': _f32(inp['dec_w']),
            'wqkv': _bf(wqkv),
            'pm': _bf(pmT),
            'wo': _bf(woA),
            'w1': _bf(w1A),
            'w2': _bf(w2A),
            'colbias': colbias,
            'rowb': _bf(rowb.reshape(1, -1)),
            'vbne': _f32(vbne.reshape(1, -1)),
        })
    return maps


_PROG_CACHE = {}


def _get_program(n_tokens=NTOK, n_cores=8):
    key = (n_tokens, n_cores)
    if key not in _PROG_CACHE:
        _PROG_CACHE[key] = build_program(n_tokens, n_cores)
    return _PROG_CACHE[key]


def kernel(**inputs):
    nc = _get_program()
    in_maps = host_prep(inputs)
    res = run_bass_kernel_spmd(nc, in_maps, list(range(8)))
    out = np.stack([res.results[2 * b]['out'][0] for b in range(B)])
    return out.astype(np.float32)


if __name__ == '__main__':
    import pickle
    inp = pickle.load(open('/root/problem/inputs_cache.pkl', 'rb'))
    inp.pop('_ref_jax', None)
    o = kernel(**inp)
    print(o.shape, o.dtype)
